# revision 10
# baseline (speedup 1.0000x reference)
"""2-layer bidirectional LSTM encoder on 8 Trainium2 NeuronCores.

Strategy (v1): pure data-parallel. Batch 64 is split 8 ways; every core runs
the complete 2-layer biLSTM on its 8 sequences. All matmuls use float32r
(full-rate fp32 on the PE at moving-dim >= 256).

Per (layer, dir) unit on each core:
  phase X: input projection xg[t,b,:] = x[b,t,:] @ Wih^T + (b_ih+b_hh)
           as big GEMMs over 128-token blocks (stationary = x^T chunks,
           moving = Wih^T), xg staged through DRAM.
  phase R: time recurrence. gates[b, 4H] accumulate in PSUM as
           hT(stationary [128,B]) x WhhT(moving [128,512] chunks), plus the
           DVE-add of xg[t]; ACT sigmoid/tanh; DVE cell update; PE transpose
           of h back to hT for the next step.

Layer-0 h states are accumulated transposed ([feat, t] per batch) in SBUF and
flushed to DRAM every 128 steps, forming layer-1's input in exactly the
layout the xproj stationary wants. Gate order is repacked on the host from
PyTorch's [i,f,g,o] to [i,f,o,g] so one ACT op covers all three sigmoids.
"""
import numpy as np

import concourse.bacc as bacc
import concourse.mybir as mybir
from concourse.tile import TileContext
from concourse.bass_utils import run_bass_kernel_spmd

F32 = mybir.dt.float32
F32R = mybir.dt.float32r
BF16 = mybir.dt.bfloat16
AF = mybir.ActivationFunctionType

NCORES = 8
B = 8          # batch per core
T = 512        # sequence length
I = 1024       # input width (= 2H, all layers)
H = 512        # hidden
G = 4 * H      # gates width 2048
KI = I // 128  # 8 k-chunks for xproj
KH = H // 128  # 4 k-chunks for recurrence
NN = G // 512  # 4 n-chunks of 512
TC = T // 128  # 4 time chunks


def _xproj(nc, sb, ps, src, wih_sb, bias_sb, xg_dram):
    """xg[t,b,:] = x[b,t,:] @ Wih^T + bias for all (t,b), 128-token blocks.

    src: DRAM [TC, B, I, 128] (x transposed per time-chunk).
    wih_sb: SBUF [128, KI*G] f32r resident. bias_sb: SBUF [1, G].
    xg_dram: DRAM [T, B, G].
    """
    for tc_i in range(TC):
        for b in range(B):
            stage = sb.tile([128, I], F32, tag="xstage", bufs=2)
            nc.sync.dma_start(
                out=stage[:].rearrange("p (k t) -> p k t", k=KI),
                in_=src[tc_i, b].rearrange("(k p) t -> p k t", p=128),
            )
            xt = sb.tile([128, I], F32R, tag="xt", bufs=2)
            nc.vector.tensor_copy(xt[:], stage[:])
            for n in range(NN):
                psum = ps.tile([128, 512], F32, tag="gx")
                for k in range(KI):
                    nc.tensor.matmul(
                        psum[:],
                        xt[:, k * 128:(k + 1) * 128],
                        wih_sb[:, k * G + n * 512:k * G + (n + 1) * 512],
                        start=(k == 0),
                        stop=(k == KI - 1),
                    )
                xgs = sb.tile([128, 512], F32, tag="xgs", bufs=3)
                nc.vector.tensor_tensor(
                    out=xgs[:],
                    in0=psum[:],
                    in1=bias_sb[:, n * 512:(n + 1) * 512],
                    op=mybir.AluOpType.add,
                )
                nc.sync.dma_start(
                    out=xg_dram[tc_i * 128:(tc_i + 1) * 128, b,
                                n * 512:(n + 1) * 512],
                    in_=xgs[:],
                )


def _recurrence(nc, sb, ps, whh_sb, ident_sb, zeros_dram, xg_dram, fwd, haccp=None,
                hcat_dram=None, hcat_off=0, out_dram=None, out_off=0):
    """One (layer, dir) LSTM scan over T steps for B local sequences.

    whh_sb: SBUF [128, KH*G] f32r. xg_dram: [T, B, G].
    hcat_dram: if set, DRAM [TC, B, 2H, 128]; h^T is accumulated in SBUF and
    flushed there every 128 steps at feature offset hcat_off (layer 0).
    out_dram: if set, DRAM [B, T, 2H]; h is written per step at feature
    offset out_off (layer 1).
    Gate order [i, f, o, g].
    """
    hT = sb.tile([128, KH * B], F32R, tag="hT", bufs=1)      # stationary, [128, 32]
    c = sb.tile([B, H], F32, tag="c", bufs=1)
    nc.sync.dma_start(out=hT[:], in_=zeros_dram[:].bitcast(F32R))
    nc.vector.memset(c[:], 0.0)
    if hcat_dram is not None:
        hacc = haccp.tile([128, KH * B * 128], F32, tag="hacc")  # (f, b, tloc)

    for s in range(T):
        t = s if fwd else T - 1 - s
        xg = sb.tile([B, G], F32, tag="xg", bufs=2)
        nc.sync.dma_start(out=xg[:], in_=xg_dram[t])
        gates = sb.tile([B, G], F32, tag="gates", bufs=2)
        for n in range(NN):
            psum = ps.tile([B, 512], F32, tag="g")
            for k in range(KH):
                nc.tensor.matmul(
                    psum[:],
                    hT[:, k * B:(k + 1) * B],
                    whh_sb[:, k * G + n * 512:k * G + (n + 1) * 512],
                    start=(k == 0),
                    stop=(k == KH - 1),
                )
            nc.vector.tensor_tensor(
                out=gates[:, n * 512:(n + 1) * 512],
                in0=psum[:],
                in1=xg[:, n * 512:(n + 1) * 512],
                op=mybir.AluOpType.add,
            )
        # activations: [i f o] sigmoid in one op, g tanh
        nc.scalar.activation(gates[:, 0:1536], gates[:, 0:1536], AF.Sigmoid)
        nc.scalar.activation(gates[:, 1536:2048], gates[:, 1536:2048], AF.Tanh)
        # c = f*c + i*g ; h = o * tanh(c)
        ig = sb.tile([B, H], F32, tag="ig", bufs=2)
        nc.vector.tensor_tensor(out=ig[:], in0=gates[:, 0:512],
                                in1=gates[:, 1536:2048], op=mybir.AluOpType.mult)
        nc.vector.tensor_tensor(out=c[:], in0=c[:], in1=gates[:, 512:1024],
                                op=mybir.AluOpType.mult)
        nc.vector.tensor_tensor(out=c[:], in0=c[:], in1=ig[:],
                                op=mybir.AluOpType.add)
        th = sb.tile([B, H], F32, tag="th", bufs=2)
        nc.scalar.activation(th[:], c[:], AF.Tanh)
        h = sb.tile([B, H], F32, tag="h", bufs=2)
        nc.vector.tensor_tensor(out=h[:], in0=gates[:, 1024:1536], in1=th[:],
                                op=mybir.AluOpType.mult)
        if out_dram is not None:
            nc.sync.dma_start(out=out_dram[:, t, out_off:out_off + H], in_=h[:])
        # h -> hT for next step (and into the transposed accumulator)
        tp = ps.tile([128, KH * B], F32, tag="tp")
        for k in range(KH):
            nc.tensor.transpose(tp[:, k * B:(k + 1) * B],
                                h[:, k * 128:(k + 1) * 128], ident_sb[:])
        nc.vector.tensor_copy(hT[:], tp[:])
        if hcat_dram is not None:
            tloc = t % 128
            nc.vector.tensor_copy(
                hacc[:].rearrange("p (k b t) -> p k b t", k=KH, b=B)[:, :, :, tloc],
                tp[:].rearrange("p (k b) -> p k b", k=KH),
            )
            if s % 128 == 127:
                cg = t // 128
                for k in range(KH):
                    nc.sync.dma_start(
                        out=hcat_dram[cg][:, hcat_off + k * 128:
                                          hcat_off + (k + 1) * 128, :]
                            .rearrange("b p t -> p b t"),
                        in_=hacc[:, k * B * 128:(k + 1) * B * 128]
                            .rearrange("p (b t) -> p b t", b=B),
                    )


def build_nc():
    nc = bacc.Bacc()
    xt_in = nc.dram_tensor("xt", [TC, B, I, 128], F32, kind="ExternalInput")
    ident = nc.dram_tensor("ident", [B, B], F32, kind="ExternalInput")
    zeros = nc.dram_tensor("zeros", [128, KH * B], F32, kind="ExternalInput")
    wih = {}
    whh = {}
    bias = {}
    for l in range(2):
        for d in range(2):
            wih[l, d] = nc.dram_tensor(f"wih{l}{d}", [128, KI * G], F32,
                                       kind="ExternalInput")
            whh[l, d] = nc.dram_tensor(f"whh{l}{d}", [128, KH * G], F32,
                                       kind="ExternalInput")
            bias[l, d] = nc.dram_tensor(f"bias{l}{d}", [128, G], F32,
                                        kind="ExternalInput")
    out = nc.dram_tensor("out", [B, T, 2 * H], F32, kind="ExternalOutput")

    xg = {(l, d): nc.dram_tensor(f"xg{l}{d}", [T, B, G], F32)
          for l in range(2) for d in range(2)}
    hcat = nc.dram_tensor("hcat", [TC, B, 2 * H, 128], F32)

    with TileContext(nc) as tc:
        with (
            tc.tile_pool(name="wpool", bufs=1) as wp,
            tc.tile_pool(name="wihp", bufs=1) as wihp,
            tc.tile_pool(name="whhp", bufs=1) as whhp,
            tc.tile_pool(name="haccp", bufs=1) as haccp,
            tc.tile_pool(name="sb", bufs=3) as sb,
            tc.tile_pool(name="ps", bufs=2, space="PSUM") as ps,
        ):
            ident_sb = wp.tile([B, B], F32, tag="ident")
            nc.sync.dma_start(out=ident_sb[:], in_=ident[:])

            for l in range(2):
                src = xt_in if l == 0 else hcat
                # --- input projections for both directions of this layer ---
                for d in range(2):
                    wih_sb = wihp.tile([128, KI * G], F32R, tag="wih")
                    nc.sync.dma_start(out=wih_sb[:], in_=wih[l, d][:].bitcast(F32R))
                    nc.tensor.ldweights(wih_sb[:, 0:1].bitcast(BF16))
                    bias_sb = sb.tile([128, G], F32, tag="biasr", bufs=2)
                    nc.sync.dma_start(out=bias_sb[:], in_=bias[l, d][:])
                    _xproj(nc, sb, ps, src, wih_sb, bias_sb, xg[l, d])
                # --- recurrences ---
                for d in range(2):
                    whh_sb = whhp.tile([128, KH * G], F32R, tag="whh")
                    nc.sync.dma_start(out=whh_sb[:], in_=whh[l, d][:].bitcast(F32R))
                    nc.tensor.ldweights(whh_sb[:, 0:1].bitcast(BF16))
                    _recurrence(
                        nc, sb, ps, whh_sb, ident_sb, zeros, xg[l, d],
                        fwd=(d == 0),
                        haccp=haccp,
                        hcat_dram=hcat if l == 0 else None, hcat_off=d * H,
                        out_dram=out if l == 1 else None, out_off=d * H,
                    )
    nc.finalize()
    return nc


_NC = None


def _get_nc():
    global _NC
    if _NC is None:
        _NC = build_nc()
    return _NC


def _pack_gates(w):
    """[4H, ...] pytorch gate order i,f,g,o -> i,f,o,g."""
    i, f, g, o = np.split(w, 4, axis=0)
    return np.concatenate([i, f, o, g], axis=0)


def _build_in_maps(inputs):
    input_tensor = np.asarray(inputs["input_tensor"], dtype=np.float32)
    W_ih = np.asarray(inputs["W_ih"], dtype=np.float32)
    W_hh = np.asarray(inputs["W_hh"], dtype=np.float32)
    b_ih = np.asarray(inputs["b_ih"], dtype=np.float32)
    b_hh = np.asarray(inputs["b_hh"], dtype=np.float32)

    common = {"ident": np.eye(B, dtype=np.float32),
              "zeros": np.zeros((128, KH * B), dtype=np.float32)}
    for l in range(2):
        for d in range(2):
            wih_t = _pack_gates(W_ih[l, d]).T          # [I, G]
            common[f"wih{l}{d}"] = np.ascontiguousarray(
                wih_t.reshape(KI, 128, G).transpose(1, 0, 2).reshape(128, KI * G))
            whh_t = _pack_gates(W_hh[l, d]).T          # [H, G]
            common[f"whh{l}{d}"] = np.ascontiguousarray(
                whh_t.reshape(KH, 128, G).transpose(1, 0, 2).reshape(128, KH * G))
            common[f"bias{l}{d}"] = np.ascontiguousarray(np.tile(
                (_pack_gates(b_ih[l, d]) + _pack_gates(b_hh[l, d]))[None, :],
                (128, 1)))

    in_maps = []
    for c in range(NCORES):
        xc = input_tensor[c * B:(c + 1) * B]           # [B, T, I]
        # [TC, B, I, 128]: time-chunked transpose
        xt = np.ascontiguousarray(
            xc.reshape(B, TC, 128, I).transpose(1, 0, 3, 2))
        in_maps.append({**common, "xt": xt})
    return in_maps


def kernel(input_tensor, W_ih, W_hh, b_ih, b_hh, batch_size, max_seq_len):
    nc = _get_nc()
    in_maps = _build_in_maps({
        "input_tensor": input_tensor, "W_ih": W_ih, "W_hh": W_hh,
        "b_ih": b_ih, "b_hh": b_hh,
    })
    res = run_bass_kernel_spmd(nc, in_maps, core_ids=list(range(NCORES)))
    out = np.concatenate([res.results[c]["out"] for c in range(NCORES)], axis=0)
    return out
